# revision 1
# baseline (speedup 1.0000x reference)
"""DIN/UBM kernel: data-parallel over 8 NeuronCores (batch 256 -> 32/core).

v1: jax/XLA implementation sharded across the 8 cores via shard_map (pure
data parallel per the sharding hint; small tables/weights replicated).
"""
import numpy as np
import jax
import jax.numpy as jnp
from jax.sharding import Mesh, PartitionSpec
from jax.experimental.shard_map import shard_map
from functools import partial

B, S, F, E = 256, 50, 30, 16
PAD_NEG = jnp.float32(-(2 ** 30) + 1)
N_CORES = 8

INT_KEYS = ["request_wday", "request_hour", "request_min", "uid", "did",
            "gender", "age", "province", "vid", "aid", "cate_two", "cate_one",
            "upload_type", "upload_ts_wday", "upload_ts_hour", "upload_ts_min",
            "seq_arr", "seq_mask", "seq_len", "flow_seq_arr", "flow_seq_mask"]
FLOAT_KEYS = ["uid_tab", "did_tab", "gender_tab", "age_tab", "province_tab",
              "vid_tab", "aid_tab", "cate2_tab", "cate1_tab", "uptype_tab",
              "wday_tab", "hour_tab", "min_tab",
              "carm_W1", "carm_b1", "carm_W2", "carm_b2",
              "din_W1", "din_b1", "din_W2", "din_b2",
              "mlp_W1", "mlp_b1", "mlp_W2", "mlp_b2",
              "mlp_W3", "mlp_b3", "mlp_W4", "mlp_b4"]
BATCH_KEYS = set(INT_KEYS)  # all int tensors are batch-dim sharded


def _emb(table, idx, pad):
    return table.at[pad].set(0.0)[idx]


def _mlp2(x, W1, b1, W2, b2):
    return jax.nn.relu(x @ W1 + b1) @ W2 + b2


def _forward(ints, floats):
    (request_wday, request_hour, request_min, uid, did, gender, age, province,
     vid, aid, cate_two, cate_one, upload_type, upload_ts_wday, upload_ts_hour,
     upload_ts_min, seq_arr, seq_mask, seq_len, flow_seq_arr, flow_seq_mask) = ints
    (uid_tab, did_tab, gender_tab, age_tab, province_tab, vid_tab, aid_tab,
     cate2_tab, cate1_tab, uptype_tab, wday_tab, hour_tab, min_tab,
     carm_W1, carm_b1, carm_W2, carm_b2, din_W1, din_b1, din_W2, din_b2,
     mlp_W1, mlp_b1, mlp_W2, mlp_b2, mlp_W3, mlp_b3, mlp_W4, mlp_b4) = floats
    item_tabs = ((vid_tab, 0), (aid_tab, 0), (cate2_tab, 0), (cate1_tab, 2), (uptype_tab, 0))
    seq_emb = jnp.concatenate([_emb(t, seq_arr[..., i], p) for i, (t, p) in enumerate(item_tabs)], axis=-1)
    flow_emb = jnp.concatenate([_emb(t, flow_seq_arr[..., i], p) for i, (t, p) in enumerate(item_tabs)], axis=-1)
    b, s, f, d5 = flow_emb.shape
    seq4 = jnp.broadcast_to(seq_emb[:, :, None, :], (b, s, f, d5))
    flow_logits = _mlp2(jnp.concatenate([flow_emb, seq4], axis=-1), carm_W1, carm_b1, carm_W2, carm_b2)[..., 0]
    flow_logits = jnp.where(flow_seq_mask != 0, flow_logits, PAD_NEG)
    flow_scores = jax.nn.softmax(flow_logits, axis=-1)
    rep = jnp.einsum('bsf,bsfd->bsd', flow_scores, flow_emb)
    vid_e = _emb(vid_tab, vid, 0); aid_e = _emb(aid_tab, aid, 0); c2_e = _emb(cate2_tab, cate_two, 0)
    c1_e = _emb(cate1_tab, cate_one, 2); up_e = _emb(uptype_tab, upload_type, 0)
    target = jnp.concatenate([vid_e, aid_e, c2_e, c1_e, up_e], axis=-1)
    din_in = jnp.concatenate([rep, jnp.broadcast_to(target[:, None, :], rep.shape)], axis=-1)
    din_logits = _mlp2(din_in, din_W1, din_b1, din_W2, din_b2)[..., 0]
    din_logits = jnp.where(seq_mask != 0, din_logits, PAD_NEG)
    din_scores = jax.nn.softmax(din_logits, axis=-1)
    din_interest = jnp.einsum('bs,bsd->bd', din_scores, rep)
    mlp_input = jnp.concatenate([
        _emb(wday_tab, request_wday, 0), _emb(hour_tab, request_hour, 0), _emb(min_tab, request_min, 0),
        _emb(uid_tab, uid, 0), _emb(did_tab, did, 0), _emb(gender_tab, gender, 0),
        _emb(age_tab, age, 0), _emb(province_tab, province, 0),
        din_interest, vid_e, aid_e, c2_e, c1_e, up_e,
        _emb(wday_tab, upload_ts_wday, 0), _emb(hour_tab, upload_ts_hour, 0), _emb(min_tab, upload_ts_min, 0)], axis=-1)
    h = jax.nn.relu(mlp_input @ mlp_W1 + mlp_b1)
    h = jax.nn.relu(h @ mlp_W2 + mlp_b2)
    h = jax.nn.relu(h @ mlp_W3 + mlp_b3)
    return (h @ mlp_W4 + mlp_b4)[:, 0]


_CACHE = {}


def _build():
    if "fn" in _CACHE:
        return _CACHE["fn"]
    devices = jax.devices()[:N_CORES]
    mesh = Mesh(np.asarray(devices), ("core",))
    in_specs = (
        tuple(PartitionSpec("core") for _ in INT_KEYS),
        tuple(PartitionSpec() for _ in FLOAT_KEYS),
    )
    out_specs = PartitionSpec("core")

    def body(ints, floats):
        return _forward(ints, floats)

    fn = jax.jit(shard_map(body, mesh=mesh, in_specs=in_specs,
                           out_specs=out_specs, check_rep=False))
    _CACHE["fn"] = fn
    return fn


def kernel(**inputs):
    fn = _build()
    ints = tuple(jnp.asarray(np.asarray(inputs[k])) for k in INT_KEYS)
    floats = tuple(jnp.asarray(np.asarray(inputs[k], np.float32)) for k in FLOAT_KEYS)
    out = fn(ints, floats)
    return np.asarray(out)
